# revision 7
# baseline (speedup 1.0000x reference)
"""Fused LN + QKV + per-token head-mixing attention + output projection
for Trainium2, data-parallel over tokens across 8 NeuronCores.

Problem shapes (hardcoded): x [4, 4096, 2048], D=2048, H=16 heads, hd=128.
reference: LN -> q,k,v = xn@W+b -> scores = einsum('bshd,bsgd->bshg', q, k)/sqrt(D)
           -> softmax(g) -> context = einsum('bshg,bsgd->bshd', w, v) -> @Wo + bo.

Everything is per-token, so tokens shard freely: core c takes tokens
[c*2048, (c+1)*2048) of the flattened [16384, 2048] stream.

Per-core pipeline:
  P1  LN (bn_stats) token-major, PE-transpose -> resident xnT [128dw,16kc,2048t] (f32r)
  P2  q/k/v = Wp.T @ xnT, weight-stationary fp32r matmuls (N=512, full PE rate),
      spill qT/kT/vT [16h,128dw,2048t] to DRAM scratch.  ln gain/bias are folded
      into Wq/Wk/Wv/biases on the host.
  P3  attention in 32-token PSUM banks; 8-token groups batched into [128,128]
      matmuls via the row/col map p = a*32 + j*16 + head (token t = 8G+2a+j):
        S^T = k_ilv.T @ q_ilv   (cross-token entries masked later)
        E = exp(S^T/sqrt(D)); den = BD16.T @ E; A^T = E * mask/den
        ctxT = vH.T @ A^T  with vH = PE-transpose(v_ilv)
      ctxT banks drain into [128dw,16h,256t] tiles -> DRAM scratch.
  P4  out^T = Wo.T @ ctxT (fp32r), +bo, PE-transpose back to token-major, DMA out.
"""
import sys

sys.path.insert(0, "/opt/trn_rl_repo")

from contextlib import ExitStack

import numpy as np

import concourse.bass as bass
import concourse.tile as tile
from concourse import bacc, mybir
from concourse.bass_utils import run_bass_kernel_spmd

F32 = mybir.dt.float32
F32R = mybir.dt.float32r
AF = mybir.ActivationFunctionType

D = 2048
H = 16
HD = 128
KC = 16              # D / 128 contraction chunks
TPC = 2048           # tokens per core
NCORES = 8
LN_EPS = 1e-5
GRP = 256            # attention group (tokens)
NGRP = TPC // GRP    # 8
NBANK = GRP // 32    # 8 banks of 32 tokens per group

_CACHED = {}


def _build_nc():
    nc = bacc.Bacc(None, target_bir_lowering=False)

    x = nc.declare_dram_parameter("x", [TPC, D], F32, isOutput=False)
    ws = {p: nc.declare_dram_parameter(f"W{p}", [D, D], F32, isOutput=False)
          for p in ("q", "k", "v", "o")}
    bs = {p: nc.declare_dram_parameter(f"b{p}", [D], F32, isOutput=False)
          for p in ("q", "k", "v", "o")}
    ident = nc.declare_dram_parameter("ident", [128, 128], F32, isOutput=False)
    bd16 = nc.declare_dram_parameter("bd16", [128, 128], F32, isOutput=False)
    mask = nc.declare_dram_parameter("mask", [128, 512], F32, isOutput=False)
    out = nc.declare_dram_parameter("out", [TPC, D], F32, isOutput=True)

    with tile.TileContext(nc) as tc, ExitStack() as top:
        const = top.enter_context(tc.tile_pool(name="const", bufs=1))
        dram = top.enter_context(tc.tile_pool(name="dram", bufs=1, space="DRAM"))

        ident_t = const.tile([128, 128], F32R)
        nc.sync.dma_start(out=ident_t, in_=ident[:, :].bitcast(F32R))
        bd16_t = const.tile([128, 128], F32R)
        nc.sync.dma_start(out=bd16_t, in_=bd16[:, :].bitcast(F32R))
        mask_t = const.tile([128, 512], F32)
        nc.sync.dma_start(out=mask_t, in_=mask[:, :])
        # per-feature biases as [128, 16] columns (col h = b[h*128:(h+1)*128])
        eps_t = const.tile([128, 1], F32)
        nc.vector.memset(eps_t, LN_EPS)
        bias_t = {}
        for p in ("q", "k", "v", "o"):
            bt = const.tile([128, H], F32, name=f"bias_{p}", tag=f"bias_{p}")
            nc.sync.dma_start(out=bt, in_=bs[p][:].rearrange("(h p) -> p h", p=128))
            bias_t[p] = bt

        # DRAM scratch, layout [head/kc, dw, t]
        scr = {p: dram.tile([H, 128, TPC], F32, name=f"scr_{p}") for p in ("q", "k", "v")}
        ctx_scr = dram.tile([H, 128, TPC], F32)

        # ---------------- P1 + P2 ----------------
        with ExitStack() as ph:
            xnt_pool = ph.enter_context(tc.tile_pool(name="xnt", bufs=1))
            p1 = ph.enter_context(tc.tile_pool(name="p1", bufs=2))
            p1ps = ph.enter_context(tc.tile_pool(name="p1ps", bufs=4, space="PSUM"))

            xnT = xnt_pool.tile([128, KC, TPC], F32R)

            for it in range(TPC // 128):
                xt = p1.tile([128, D], F32, tag="xt")
                nc.sync.dma_start(out=xt, in_=x[it * 128:(it + 1) * 128, :])
                stats = p1.tile([128, 4, 6], F32, tag="stats")
                for i in range(4):
                    nc.vector.bn_stats(out=stats[:, i, :],
                                       in_=xt[:, i * 512:(i + 1) * 512])
                mv = p1.tile([128, 2], F32, tag="mv")
                nc.vector.bn_aggr(out=mv, in_=stats)
                rstd = p1.tile([128, 1], F32, tag="rstd")
                nc.scalar.activation(out=rstd, in_=mv[:, 1:2], func=AF.Sqrt,
                                     bias=eps_t, scale=1.0)
                nc.vector.reciprocal(out=rstd, in_=rstd)
                xn = p1.tile([128, D], F32R, tag="xn")
                nc.vector.tensor_scalar(out=xn, in0=xt, scalar1=mv[:, 0:1],
                                        scalar2=rstd,
                                        op0=mybir.AluOpType.subtract,
                                        op1=mybir.AluOpType.mult)
                for kc in range(KC):
                    tp = p1ps.tile([128, 128], F32R, tag="tp")
                    nc.tensor.transpose(out=tp, in_=xn[:, kc * 128:(kc + 1) * 128],
                                        identity=ident_t)
                    nc.scalar.copy(out=xnT[:, kc, it * 128:(it + 1) * 128], in_=tp)

            # P2: weight-stationary projections
            p2w = ph.enter_context(tc.tile_pool(name="p2w", bufs=2))
            p2s = ph.enter_context(tc.tile_pool(name="p2s", bufs=4))
            p2ps = ph.enter_context(tc.tile_pool(name="p2ps", bufs=4, space="PSUM"))
            for p in ("q", "k", "v"):
                for h in range(H):
                    wp = p2w.tile([128, KC, 128], F32R, tag="wp")
                    nc.sync.dma_start(
                        out=wp,
                        in_=ws[p][:, h * 128:(h + 1) * 128]
                        .rearrange("(kc p) n -> p kc n", p=128).bitcast(F32R))
                    for tg in range(4):
                        bank = p2ps.tile([128, 512], F32, tag="bank")
                        for kc in range(KC):
                            nc.tensor.matmul(
                                out=bank, lhsT=wp[:, kc, :],
                                rhs=xnT[:, kc, tg * 512:(tg + 1) * 512],
                                start=(kc == 0), stop=(kc == KC - 1))
                        stage = p2s.tile([128, 512], F32, tag="stage")
                        nc.vector.tensor_scalar_add(out=stage, in0=bank,
                                                    scalar1=bias_t[p][:, h:h + 1])
                        nc.sync.dma_start(
                            out=scr[p][h, :, tg * 512:(tg + 1) * 512], in_=stage)

        # ---------------- P3: attention ----------------
        with ExitStack() as ph:
            qkv = ph.enter_context(tc.tile_pool(name="qkv", bufs=2))
            ilv = ph.enter_context(tc.tile_pool(name="ilv", bufs=3))
            sfm = ph.enter_context(tc.tile_pool(name="sfm", bufs=2))
            cts = ph.enter_context(tc.tile_pool(name="cts", bufs=2))
            aps = ph.enter_context(tc.tile_pool(name="aps", bufs=2, space="PSUM"))

            for g in range(NGRP):
                t0 = g * GRP
                qg = qkv.tile([128, H, GRP], F32R, tag="qg")
                kg = qkv.tile([128, H, GRP], F32R, tag="kg")
                vg = qkv.tile([128, H, GRP], F32R, tag="vg")
                for t, p in ((qg, "q"), (kg, "k"), (vg, "v")):
                    nc.sync.dma_start(
                        out=t,
                        in_=scr[p][:, :, t0:t0 + GRP]
                        .rearrange("h p t -> p h t").bitcast(F32R))
                ctxT = cts.tile([128, H, GRP], F32, tag="ctxT")

                for b in range(NBANK):
                    w0 = b * 32
                    s_ps = aps.tile([128, 512], F32, tag="s")
                    ilvs = []
                    for G in range(4):
                        qi = ilv.tile([128, 128], F32R, tag="qi")
                        nc.scalar.copy(
                            out=qi.rearrange("p (a j h) -> p a j h", a=4, j=2),
                            in_=qg[:, :, w0 + 8 * G:w0 + 8 * G + 8]
                            .rearrange("p h (a j) -> p a j h", a=4))
                        ki = ilv.tile([128, 128], F32R, tag="ki")
                        nc.vector.tensor_copy(
                            out=ki.rearrange("p (a j h) -> p a j h", a=4, j=2),
                            in_=kg[:, :, w0 + 8 * G:w0 + 8 * G + 8]
                            .rearrange("p h (a j) -> p a j h", a=4))
                        vi = ilv.tile([128, 128], F32R, tag="vi")
                        nc.gpsimd.tensor_copy(
                            out=vi.rearrange("p (a j h) -> p a j h", a=4, j=2),
                            in_=vg[:, :, w0 + 8 * G:w0 + 8 * G + 8]
                            .rearrange("p h (a j) -> p a j h", a=4))
                        nc.tensor.matmul(out=s_ps[:, 128 * G:128 * (G + 1)],
                                         lhsT=ki, rhs=qi, start=True, stop=True)
                        ilvs.append(vi)

                    e_sb = sfm.tile([128, 512], F32R, tag="e")
                    nc.scalar.activation(out=e_sb, in_=s_ps, func=AF.Exp,
                                         scale=float(1.0 / np.sqrt(D)))
                    den_ps = aps.tile([128, 512], F32, tag="den")
                    nc.tensor.matmul(out=den_ps, lhsT=bd16_t, rhs=e_sb,
                                     start=True, stop=True)
                    r_sb = sfm.tile([128, 512], F32, tag="r")
                    nc.vector.reciprocal(out=r_sb, in_=den_ps)
                    rm_sb = sfm.tile([128, 512], F32, tag="rm")
                    nc.vector.tensor_mul(out=rm_sb, in0=r_sb, in1=mask_t)
                    at_sb = sfm.tile([128, 512], F32R, tag="at")
                    nc.vector.tensor_mul(out=at_sb, in0=e_sb, in1=rm_sb)

                    ctx_ps = aps.tile([128, 512], F32, tag="ctx")
                    for G in range(4):
                        vh_ps = aps.tile([128, 128], F32R, tag="vh")
                        nc.tensor.transpose(out=vh_ps, in_=ilvs[G],
                                            identity=ident_t)
                        vh_sb = ilv.tile([128, 128], F32R, tag="vhs")
                        nc.vector.tensor_copy(out=vh_sb, in_=vh_ps)
                        nc.tensor.matmul(out=ctx_ps[:, 128 * G:128 * (G + 1)],
                                         lhsT=vh_sb,
                                         rhs=at_sb[:, 128 * G:128 * (G + 1)],
                                         start=True, stop=True)
                    nc.scalar.copy(
                        out=ctxT[:, :, w0:w0 + 32]
                        .rearrange("p h (G a j) -> p G a j h", G=4, a=4),
                        in_=ctx_ps.rearrange("p (G a j h) -> p G a j h",
                                             G=4, a=4, j=2))

                nc.sync.dma_start(
                    out=ctx_scr[:, :, t0:t0 + GRP].rearrange("h p t -> p h t"),
                    in_=ctxT)

        # ---------------- P4: output projection ----------------
        with ExitStack() as ph:
            cta = ph.enter_context(tc.tile_pool(name="cta", bufs=1))
            p4w = ph.enter_context(tc.tile_pool(name="p4w", bufs=3))
            p4s = ph.enter_context(tc.tile_pool(name="p4s", bufs=4))
            p4o = ph.enter_context(tc.tile_pool(name="p4o", bufs=4))
            p4ps = ph.enter_context(tc.tile_pool(name="p4ps", bufs=3, space="PSUM"))
            p4tp = ph.enter_context(tc.tile_pool(name="p4tp", bufs=3, space="PSUM"))

            ctxA = cta.tile([128, KC, TPC], F32R)
            nc.sync.dma_start(
                out=ctxA,
                in_=ctx_scr[:, :, :].rearrange("h p t -> p h t").bitcast(F32R))

            for h in range(H):
                wp = p4w.tile([128, KC, 128], F32R, tag="wp")
                nc.sync.dma_start(
                    out=wp,
                    in_=ws["o"][:, h * 128:(h + 1) * 128]
                    .rearrange("(kc p) n -> p kc n", p=128).bitcast(F32R))
                for tg in range(4):
                    bank = p4ps.tile([128, 512], F32, tag="bank")
                    for kc in range(KC):
                        nc.tensor.matmul(
                            out=bank, lhsT=wp[:, kc, :],
                            rhs=ctxA[:, kc, tg * 512:(tg + 1) * 512],
                            start=(kc == 0), stop=(kc == KC - 1))
                    stage = p4s.tile([128, 512], F32R, tag="stage")
                    nc.vector.tensor_scalar_add(out=stage, in0=bank,
                                                scalar1=bias_t["o"][:, h:h + 1])
                    for s in range(4):
                        tp = p4tp.tile([128, 128], F32R, tag="tp")
                        nc.tensor.transpose(out=tp,
                                            in_=stage[:, s * 128:(s + 1) * 128],
                                            identity=ident_t)
                        ot = p4o.tile([128, 128], F32, tag="ot")
                        nc.scalar.copy(out=ot, in_=tp)
                        trow = tg * 512 + s * 128
                        nc.sync.dma_start(
                            out=out[trow:trow + 128, h * 128:(h + 1) * 128],
                            in_=ot)

    nc.finalize()
    return nc


def _constants():
    ident = np.eye(128, dtype=np.float32)
    bd16 = np.kron(np.eye(8, dtype=np.float32),
                   np.ones((16, 16), np.float32))
    r = np.arange(128)
    c = np.arange(512)
    mask = ((r[:, None] // 32 == (c[None, :] % 128) // 32)
            & ((r[:, None] // 16) % 2 == ((c[None, :] % 128) // 16) % 2)
            ).astype(np.float32)
    return ident, bd16, mask


def kernel(x, ln_g, ln_b, Wq, bq, Wk, bk, Wv, bv, Wo, bo):
    x = np.asarray(x, dtype=np.float32)
    B, S, _ = x.shape
    xt = np.ascontiguousarray(x.reshape(B * S, D))

    g = np.asarray(ln_g, np.float32)
    b = np.asarray(ln_b, np.float32)
    # fold LN gain/bias into the QKV weights: (xn*g + b) @ W + bias
    folded = {}
    for p, W, bias in (("q", Wq, bq), ("k", Wk, bk), ("v", Wv, bv)):
        W = np.asarray(W, np.float32)
        bias = np.asarray(bias, np.float32)
        folded[p] = (np.ascontiguousarray(g[:, None] * W),
                     (b @ W + bias).astype(np.float32))
    folded["o"] = (np.ascontiguousarray(np.asarray(Wo, np.float32)),
                   np.asarray(bo, np.float32))

    ident, bd16, mask = _constants()

    if "nc" not in _CACHED:
        _CACHED["nc"] = _build_nc()
    nc = _CACHED["nc"]

    in_maps = []
    for cid in range(NCORES):
        m = {"x": np.ascontiguousarray(xt[cid * TPC:(cid + 1) * TPC]),
             "ident": ident, "bd16": bd16, "mask": mask}
        for p in ("q", "k", "v", "o"):
            m[f"W{p}"], m[f"b{p}"] = folded[p]
        in_maps.append(m)

    res = run_bass_kernel_spmd(nc, in_maps, list(range(NCORES)))
    shards = [res.results[cid]["out"] for cid in range(NCORES)]
    return np.concatenate(shards, axis=0).reshape(B, S, D)
